# revision 12
# baseline (speedup 1.0000x reference)
"""Differential multi-headed attention on 8 Trainium2 NeuronCores.

Sharding: core c = (batch b = c // 2, head-group g = c % 2).  Each core
computes 4 of the 8 differential heads for one batch element, including
the Q/K/V projections restricted to its 512 output dims, the differential
attention, and a partial output projection.  The host sums the two
partial outputs per batch (the "all-reduce"), un-permutes rows, and adds
the output bias.

Device schedule notes (v3):
 - The TRN2 PE has a p-state ramp (0.65 -> 1.2 -> 2.4 GHz over 3us of
   *continuous* execution; any idle gap resets it).  The kernel is a
   software pipeline that keeps the PE stream gap-free: projection
   matmuls for later heads are interleaved as filler between the
   attention matmuls of the current head (V of head j runs just-in-time
   inside head j itself, so even the last head has filler).
 - Input DMAs are issued from the GPSIMD queue (25ns/issue vs 565ns on
   SP), in consumption order, with x tiles split into column halves so
   the d-progressive projection groups start as soon as pieces land.
 - The softmax key-sum (fold tree + ones-matmul partition reduction) for
   one half is issued *after* the next half's first score chunks, so the
   PE never waits on the DVE/GPSIMD fold tree.  Reciprocals and the
   differential combine are issued only once the DRAM broadcast bounce
   has landed (combine split DVE/GPSIMD across query halves).
 - PSUM budget (8 banks): score ring 2x[128,1024] (4), AV accumulator
   [128,1024] (2), scratch (projections+sums) 2x[128,512] (2).
"""

import math
from contextlib import ExitStack

import ml_dtypes
import numpy as np

import concourse.bass as bass
import concourse.mybir as mybir
from concourse import bacc
import concourse.tile as tile
from concourse.bass_utils import run_bass_kernel_spmd

F32 = mybir.dt.float32
BF16 = mybir.dt.bfloat16
AF = mybir.ActivationFunctionType
ALU = mybir.AluOpType

N = 1024          # sequence length
D = 1024          # model dim
HG = 512          # head-group dims per core (4 heads x 128)
NHEAD = 4         # local heads per core
SCALE = 1.0 / math.sqrt(64.0)   # 1/sqrt(dk/2)
LAMBDA_INIT = 0.8

_BUILT = None     # cached Bass module -- building + compiling is expensive
LAST_RESULT = None  # BassKernelResults from the most recent run (for test.py)


def _build():
    nc = bacc.Bacc()

    # ---- DRAM I/O (per core) ----
    xqT = nc.dram_tensor("xqT", [D, N], BF16, kind="ExternalInput")
    xkT = nc.dram_tensor("xkT", [D, N], BF16, kind="ExternalInput")
    xvT = nc.dram_tensor("xvT", [D, N], BF16, kind="ExternalInput")  # col-permuted
    wqT = nc.dram_tensor("wqT", [D, HG], BF16, kind="ExternalInput")
    wkT = nc.dram_tensor("wkT", [D, HG], BF16, kind="ExternalInput")
    wvT = nc.dram_tensor("wvT", [D, HG], BF16, kind="ExternalInput")
    woT = nc.dram_tensor("woT", [HG, D], BF16, kind="ExternalInput")
    bq = nc.dram_tensor("bq", [HG], F32, kind="ExternalInput")
    bk = nc.dram_tensor("bk", [HG], F32, kind="ExternalInput")
    bv = nc.dram_tensor("bv", [HG], F32, kind="ExternalInput")
    lamneg = nc.dram_tensor("lamneg", [1, 1], F32, kind="ExternalInput")
    out = nc.dram_tensor("out", [N, D], F32, kind="ExternalOutput")

    with tile.TileContext(nc) as tc, ExitStack() as ctx:
        const = ctx.enter_context(tc.tile_pool(name="const", bufs=1))
        ones_bf = const.tile([128, 1], BF16, name="ones_bf")
        nc.vector.memset(ones_bf[:], 1.0)
        lamneg_bc = const.tile([128, 1], F32, name="lamneg_bc")
        bq_sb = const.tile([128, 4], F32, name="bq_sb")
        bk_sb = const.tile([128, 4], F32, name="bk_sb")
        bv_sb = const.tile([128, HG], F32, name="bv_sb")

        # Persistent activations / weights
        persist = ctx.enter_context(tc.tile_pool(name="persist", bufs=1))
        qt = [persist.tile([128, N], BF16, name=f"qt{t}") for t in range(4)]
        # qsw[j]: partitions 0:64 = qt[j][64:128], 64:128 = qt[j][0:64] -- lets
        # every (w, u) scores matmul read lhsT and rhs from the same base
        # partition.
        qsw = [persist.tile([128, N], BF16, name=f"qsw{t}") for t in range(4)]
        kt = [persist.tile([128, N], BF16, name=f"kt{t}") for t in range(4)]
        vv = [persist.tile([128, HG], BF16, name=f"vv{m}") for m in range(8)]
        oh = [persist.tile([128, N], BF16, name=f"oh{j}") for j in range(NHEAD)]
        wo_sb = [persist.tile([128, N], BF16, name=f"wo{t}") for t in range(4)]

        xw = ctx.enter_context(tc.tile_pool(name="xw", bufs=1))
        xs = {}
        ws = {}
        for nm in ("q", "k", "v"):
            for d in range(8):
                xst = xw.tile([128, N], BF16, name=f"x{nm}{d}")
                xs[(nm, d)] = xst
                wst = xw.tile([128, HG], BF16, name=f"w{nm}{d}")
                ws[(nm, d)] = wst

        # Input DMAs, in consumption order, spread across issue queues:
        # SP takes the big x tiles, ACT the weights, DVE the small consts
        # and wo -- serial per-issue cost (~0.6us) would otherwise delay
        # the later tiles by ~30us.
        nc.gpsimd.dma_start(out=bq_sb[:], in_=bq.rearrange("(t p) -> p t", p=128))
        nc.gpsimd.dma_start(out=bk_sb[:], in_=bk.rearrange("(t p) -> p t", p=128))
        nc.gpsimd.dma_start(out=bv_sb[:], in_=bv[None, :].to_broadcast([128, HG]))
        nc.gpsimd.dma_start(out=lamneg_bc[:],
                            in_=lamneg[0, :].partition_broadcast(128))
        for t in range(4):
            nc.gpsimd.dma_start(out=wo_sb[t][:],
                                in_=woT[t * 128:(t + 1) * 128, :])
        for nm, xd, wd in (("q", xqT, wqT), ("k", xkT, wkT), ("v", xvT, wvT)):
            for d in range(8):
                nc.sync.dma_start(out=xs[(nm, d)][:],
                                  in_=xd[d * 128:(d + 1) * 128, :])
                nc.scalar.dma_start(out=ws[(nm, d)][:],
                                    in_=wd[d * 128:(d + 1) * 128, :])

        # Working pools
        etp = ctx.enter_context(tc.tile_pool(name="etp", bufs=9))
        coefp = ctx.enter_context(tc.tile_pool(name="coefp", bufs=4))
        fap = ctx.enter_context(tc.tile_pool(name="fap", bufs=3))
        ffp = ctx.enter_context(tc.tile_pool(name="ffp", bufs=2))
        bcastp = ctx.enter_context(tc.tile_pool(name="bcastp", bufs=3))
        drb = ctx.enter_context(tc.tile_pool(name="drb", bufs=4, space="DRAM"))
        osb = ctx.enter_context(tc.tile_pool(name="osb", bufs=3))
        tmpp = ctx.enter_context(tc.tile_pool(name="tmpp", bufs=1))
        ostg = ctx.enter_context(tc.tile_pool(name="ostg", bufs=2))

        # PSUM pools -- exactly 8 banks
        psp = ctx.enter_context(tc.tile_pool(name="psp", bufs=2, space="PSUM"))
        pops = ctx.enter_context(tc.tile_pool(name="pops", bufs=1, space="PSUM"))
        pscr = ctx.enter_context(tc.tile_pool(name="pscr", bufs=2, space="PSUM"))

        # ---- projection work generators (PE filler units) ----
        def qk_units(j, nm):
            dst = qt[j] if nm == "q" else kt[j]
            bias = bq_sb if nm == "q" else bk_sb
            for half in range(2):
                ps = pscr.tile([128, 512], F32, name=f"p{nm}{j}{half}", tag="scr")
                for d in range(8):
                    def mm(d=d, ps=ps, nm=nm, half=half, j=j):
                        nc.tensor.matmul(
                            ps[:],
                            ws[(nm, d)][:, j * 128:(j + 1) * 128],
                            xs[(nm, d)][:, half * 512:(half + 1) * 512],
                            start=(d == 0), stop=(d == 7),
                        )
                    yield ("mm", mm)

                def post(ps=ps, dst=dst, bias=bias, half=half, j=j, nm=nm):
                    nc.vector.tensor_scalar_add(
                        dst[:, half * 512:(half + 1) * 512], ps[:],
                        bias[:, j:j + 1],
                    )
                    if nm == "q":
                        nc.sync.dma_start(
                            out=qsw[j][0:64, half * 512:(half + 1) * 512],
                            in_=qt[j][64:128, half * 512:(half + 1) * 512])
                        nc.sync.dma_start(
                            out=qsw[j][64:128, half * 512:(half + 1) * 512],
                            in_=qt[j][0:64, half * 512:(half + 1) * 512])
                yield ("post", post)

        def v_units(j, mcs=(0, 1, 2, 3)):
            # V projection in consumption order (mi, mi+4) pairs
            for mc in mcs:
                ps = pscr.tile([128, 512], F32, name=f"pv{j}{mc}", tag="scr")
                for sub, mi in ((0, mc), (1, mc + 4)):
                    for d in range(8):
                        def mm(d=d, ps=ps, sub=sub, mi=mi, j=j):
                            nc.tensor.matmul(
                                ps[:, sub * 128:(sub + 1) * 128],
                                xs[("v", d)][:, mi * 128:(mi + 1) * 128],
                                ws[("v", d)][:, j * 128:(j + 1) * 128],
                                start=(d == 0), stop=(d == 7),
                            )
                        yield ("mm", mm)

                def post(ps=ps, mc=mc, j=j):
                    for sub, mi in ((0, mc), (1, mc + 4)):
                        nc.vector.tensor_tensor(
                            out=vv[mi][:, j * 128:(j + 1) * 128],
                            in0=ps[:, sub * 128:(sub + 1) * 128],
                            in1=bv_sb[:, j * 128:(j + 1) * 128], op=ALU.add,
                        )
                yield ("post", post)

        def chain(*gens):
            for g in gens:
                yield from g

        filler = {"gen": None}

        def pull(n):
            g = filler["gen"]
            if g is None:
                return
            cnt = 0
            while cnt < n:
                u = next(g, None)
                if u is None:
                    filler["gen"] = None
                    return
                kind, fn = u
                fn()
                if kind == "mm":
                    cnt += 1

        def drain():
            g = filler["gen"]
            if g is None:
                return
            for kind, fn in g:
                fn()
            filler["gen"] = None

        # ---- prologue: Q/K projections for heads 0 and 1 (d-progressive,
        # consuming input DMAs as they land) ----
        filler["gen"] = chain(qk_units(0, "q"), qk_units(1, "q"),
                              qk_units(0, "k"), qk_units(1, "k"))
        drain()

        # filler schedule: V of head j runs just-in-time inside head j.
        head_filler = {
            0: lambda: chain(v_units(0), v_units(1)),
            1: lambda: chain(qk_units(2, "q"), qk_units(2, "k"), v_units(2)),
            2: lambda: chain(qk_units(3, "q"), qk_units(3, "k"),
                             v_units(3, mcs=(0, 1))),
            3: lambda: v_units(3, mcs=(2, 3)),
        }
        head_pull = {0: 8, 1: 6, 2: 6, 3: 8}

        # state carried between halves for the delayed sum reduction
        pend = {"ff": None, "bcx": None, "head": None, "half": None}
        parts = {j: {} for j in range(NHEAD)}   # per-head ot/bcx tiles
        combine_q = []
        recip0_q = []

        def emit_sps(u):
            """ones-matmul partition reduction + coef copy + broadcast DMAs
            for the pending half (u-th query half)."""
            ff = pend["ff"]
            hd, hf = pend["head"], pend["half"]
            sps = pscr.tile([1, 512], F32, name=f"sps{hd}{hf}{u}", tag="scr")
            nc.tensor.matmul(
                sps[:], ones_bf[:], ff[:, u * 512:(u + 1) * 512],
                start=True, stop=True,
            )
            cfs = coefp.tile([1, 512], F32, name=f"cfs{hd}{hf}{u}", tag="coef")
            nc.scalar.copy(cfs[:], sps[:])
            drs = drb.tile([1, 512], F32, name=f"drs{hd}{hf}{u}", tag="dr")
            nc.sync.dma_start(out=drs[:], in_=cfs[:])
            bcx = pend["bcx"]
            nc.sync.dma_start(
                out=bcx[:, u * 512:(u + 1) * 512],
                in_=drs[0, :].partition_broadcast(128),
            )
            if u == 1:
                parts[hd][f"bcx{hf}"] = bcx
                pend["ff"] = None
                if hf == 0:
                    recip0_q.append(hd)
                else:
                    combine_q.append(hd)

        def emit_recip0():
            while recip0_q:
                hd = recip0_q.pop(0)
                bcx0 = parts[hd]["bcx0"]
                nc.vector.reciprocal_approx_fast(out=bcx0[:], in_=bcx0[:])

        def emit_combine():
            """Recip of the second half + differential combine, split
            across DVE (u=0) and GPSIMD (u=1)."""
            if not combine_q:
                return
            hd = combine_q.pop(0)
            st = parts[hd]
            ot0, ot1 = st["ot0"], st["ot1"]
            bcx0, bcx1 = st["bcx0"], st["bcx1"]
            nc.vector.reciprocal_approx_fast(out=bcx1[:], in_=bcx1[:])
            # fold -lambda into the second-half normalizer so the combine is
            # plain tensor_tensor ops (Pool-ISA legal for the GPSIMD side)
            nc.vector.tensor_scalar_mul(bcx1[:], bcx1[:], lamneg_bc[:])
            for u in range(2):
                eng = nc.vector if u == 0 else nc.gpsimd
                sl = slice(u * 512, (u + 1) * 512)
                t1u = tmpp.tile([128, 512], F32, name=f"t1u{hd}{u}", tag=f"t1{u}")
                eng.tensor_tensor(
                    out=t1u[:], in0=ot0[:, sl], in1=bcx0[:, sl], op=ALU.mult)
                t2u = tmpp.tile([128, 512], F32, name=f"t2u{hd}{u}", tag=f"t2{u}")
                eng.tensor_tensor(
                    out=t2u[:], in0=ot1[:, sl], in1=bcx1[:, sl], op=ALU.mult)
                eng.tensor_tensor(
                    out=oh[hd][:, sl], in0=t1u[:], in1=t2u[:], op=ALU.add)

        # ---- attention per head, pipelined with projection filler ----
        for j in range(NHEAD):
            drain()
            filler["gen"] = head_filler[j]()
            for half, lo in ((0, 0), (1, 512)):
                ets = []
                ops = pops.tile([128, N], F32, name=f"ops{j}{half}", tag="ops")
                fa = [None] * 4
                fb = None

                def av(c, start, stop, j=j, ops=ops, ets=ets):
                    mi = (c % 2) * 4 + (c // 2)
                    for u in range(2):
                        nc.tensor.matmul(
                            ops[:, u * 512:(u + 1) * 512],
                            vv[mi][:, j * 128:(j + 1) * 128],
                            ets[c][:, u * 512:(u + 1) * 512],
                            start=start, stop=stop,
                        )

                for c in range(8):
                    mc, w = c // 2, c % 2
                    sp = psp.tile([128, N], F32, name=f"sp{j}{half}{c}", tag="sp")
                    for u in range(2):
                        qsrc = qt[j] if u == w else qsw[j]
                        nc.tensor.matmul(
                            sp[:, u * 512:(u + 1) * 512],
                            kt[j][w * 64:(w + 1) * 64,
                                  lo + mc * 128:lo + (mc + 1) * 128],
                            qsrc[w * 64:(w + 1) * 64, lo:lo + 512],
                            start=True, stop=True,
                        )
                    et = etp.tile([128, N], BF16, name=f"et{j}{half}{c}", tag="et")
                    nc.scalar.activation(et[:], sp[:], AF.Exp, scale=SCALE)
                    ets.append(et)

                    # delayed partition-sums of the previous half
                    if pend["ff"] is not None:
                        if c == 2:
                            emit_sps(0)
                        elif c == 5:
                            emit_sps(1)

                    pull(16 if c == 0 else head_pull[j])
                    if c >= 1:
                        av(c - 1, start=(c == 1), stop=False)

                    # fold tree: pairs (0,1) DVE, (2,3) GPSIMD, (4,5) GPSIMD,
                    # (6,7) DVE; inner combines scheduled so no engine ever
                    # head-of-line blocks on a slow producer.
                    if c == 1:
                        fa[0] = fap.tile([128, N], BF16, name=f"fa{j}{half}0",
                                         tag="fa")
                        nc.vector.tensor_tensor(out=fa[0][:], in0=ets[0][:],
                                                in1=ets[1][:], op=ALU.add)
                    elif c == 3:
                        fa[1] = fap.tile([128, N], BF16, name=f"fa{j}{half}1",
                                         tag="fa")
                        nc.gpsimd.tensor_tensor(out=fa[1][:], in0=ets[2][:],
                                                in1=ets[3][:], op=ALU.add)
                    elif c == 5:
                        fa[2] = fap.tile([128, N], BF16, name=f"fa{j}{half}2",
                                         tag="fa")
                        nc.gpsimd.tensor_tensor(out=fa[2][:], in0=ets[4][:],
                                                in1=ets[5][:], op=ALU.add)
                        fb = fap.tile([128, N], BF16, name=f"fb{j}{half}",
                                      tag="fb")
                        nc.gpsimd.tensor_tensor(out=fb[:], in0=fa[0][:],
                                                in1=fa[1][:], op=ALU.add)
                    elif c == 7:
                        fa[3] = fap.tile([128, N], BF16, name=f"fa{j}{half}3",
                                         tag="fa")
                        nc.vector.tensor_tensor(out=fa[3][:], in0=ets[6][:],
                                                in1=ets[7][:], op=ALU.add)
                        fc = fap.tile([128, N], BF16, name=f"fc{j}{half}",
                                      tag="fc")
                        nc.vector.tensor_tensor(out=fc[:], in0=fa[2][:],
                                                in1=fa[3][:], op=ALU.add)
                        ff = ffp.tile([128, N], BF16, name=f"ff{j}{half}",
                                      tag="ff")
                        nc.vector.tensor_tensor(out=ff[:], in0=fb[:],
                                                in1=fc[:], op=ALU.add)
                        emit_recip0()
                        emit_combine()

                av(7, start=False, stop=True)
                ot = osb.tile([128, N], F32, name=f"ot{j}{half}", tag="ot")
                nc.scalar.copy(ot[:], ops[:])
                parts[j][f"ot{half}"] = ot

                bcx = bcastp.tile([128, N], F32, name=f"bcx{j}{half}", tag="bc")
                pend.update(ff=ff, bcx=bcx, head=j, half=half)

        # ---- tail: last half's sums, final combine, output projection ----
        drain()
        emit_sps(0)
        emit_sps(1)

        ps_of = {}
        tags = ["sp", "sp", "ops", "scr", "scr"]
        pools = {"sp": psp, "ops": pops, "scr": pscr}

        def alloc_out_ps(idx, k):
            tg = tags[k % 5]
            ps = pools[tg].tile([128, 512], F32, name=f"po{idx}", tag=tg)
            ps_of[idx] = ps

        def outproj_mm(idx, jlist, start):
            nci, half = idx // 2, idx % 2
            ps = ps_of[idx]
            for jj in jlist:
                nc.tensor.matmul(
                    ps[:],
                    oh[jj][:, nci * 128:(nci + 1) * 128],
                    wo_sb[jj][:, half * 512:(half + 1) * 512],
                    start=(start and jj == jlist[0]), stop=(jj == 3),
                )

        def outproj_fin(idx):
            nci, half = idx // 2, idx % 2
            ps = ps_of[idx]
            stg = ostg.tile([128, 512], F32, name=f"stg{idx}", tag="og")
            nc.scalar.copy(stg[:], ps[:])
            for q in range(2):
                nc.sync.dma_start(
                    out=out[nci * 128:(nci + 1) * 128,
                            half * 512 + q * 256:half * 512 + (q + 1) * 256],
                    in_=stg[:, q * 256:(q + 1) * 256],
                )

        # wave A: pre-accumulate heads 0..2 while head 3's sum-bounce is in
        # flight.
        for k, idx in enumerate(range(5)):
            alloc_out_ps(idx, k)
            outproj_mm(idx, [0, 1, 2], start=True)
        emit_combine()
        for idx in range(5):
            outproj_mm(idx, [3], start=False)
            outproj_fin(idx)
        for k, idx in enumerate(range(5, 16)):
            alloc_out_ps(idx, 5 + k)
            outproj_mm(idx, [0, 1, 2, 3], start=True)
            outproj_fin(idx)

    if not nc.is_finalized():
        nc.finalize()
    return nc


def _get_built():
    global _BUILT
    if _BUILT is None:
        _BUILT = _build()
    return _BUILT


def kernel(**inputs):
    inp = {k: np.asarray(v) for k, v in inputs.items()}
    q_, k_, v_ = inp["query"], inp["key"], inp["value"]
    Wq, Wk, Wv, Wo = inp["Wq"], inp["Wk"], inp["Wv"], inp["Wo"]
    bq_, bk_, bv_, bo_ = inp["bq"], inp["bk"], inp["bv"], inp["bo"]
    B = q_.shape[0]

    lam = (np.exp(np.sum(inp["lambda_q1"].astype(np.float64) * inp["lambda_k1"].astype(np.float64)))
           - np.exp(np.sum(inp["lambda_q2"].astype(np.float64) * inp["lambda_k2"].astype(np.float64)))
           + LAMBDA_INIT)

    # value-row permutation: xv'[w*512 + m] = xv[2m + w]
    permv = np.arange(N).reshape(512, 2).T.reshape(-1)  # index i'=w*512+m -> 2m+w

    in_maps = []
    for c in range(8):
        b, g = c // 2, c % 2
        sl = slice(g * HG, (g + 1) * HG)
        bf = ml_dtypes.bfloat16
        in_maps.append({
            "xqT": np.ascontiguousarray(q_[b].T).astype(bf),
            "xkT": np.ascontiguousarray(k_[b].T).astype(bf),
            "xvT": np.ascontiguousarray(v_[b][permv].T).astype(bf),
            "wqT": np.ascontiguousarray(Wq[sl, :].T).astype(bf),
            "wkT": np.ascontiguousarray(Wk[sl, :].T).astype(bf),
            "wvT": np.ascontiguousarray(Wv[sl, :].T).astype(bf),
            "woT": np.ascontiguousarray(Wo[:, sl].T).astype(bf),
            "bq": np.ascontiguousarray(bq_[sl]),
            "bk": np.ascontiguousarray(bk_[sl]),
            "bv": np.ascontiguousarray(bv_[sl]),
            "lamneg": np.array([[-lam]], dtype=np.float32),
        })

    nc = _get_built()
    res = run_bass_kernel_spmd(nc, in_maps, core_ids=list(range(8)))
    global LAST_RESULT
    LAST_RESULT = res

    out = np.zeros((B, N, D), np.float32)
    for b in range(B):
        tot = res.results[2 * b]["out"] + res.results[2 * b + 1]["out"]
        # undo n' = (u, n) row order -> n2 = 2n + u
        out[b] = tot.reshape(2, 512, D).transpose(1, 0, 2).reshape(N, D) + bo_
    return out


# revision 13
# speedup vs baseline: 1.1230x; 1.1230x over previous
"""Differential multi-headed attention on 8 Trainium2 NeuronCores.

Sharding: core c = (batch b = c // 2, head-group g = c % 2).  Each core
computes 4 of the 8 differential heads for one batch element, including
the Q/K/V projections restricted to its 512 output dims, the differential
attention, and a partial output projection.  The host sums the two
partial outputs per batch (the "all-reduce"), un-permutes rows, and adds
the output bias.

Device schedule notes (v3):
 - The TRN2 PE has a p-state ramp (0.65 -> 1.2 -> 2.4 GHz over 3us of
   *continuous* execution; any idle gap resets it).  The kernel is a
   software pipeline that keeps the PE stream gap-free: projection
   matmuls for later heads are interleaved as filler between the
   attention matmuls of the current head (V of head j runs just-in-time
   inside head j itself, so even the last head has filler).
 - Input DMAs are issued from the GPSIMD queue (25ns/issue vs 565ns on
   SP), in consumption order, with x tiles split into column halves so
   the d-progressive projection groups start as soon as pieces land.
 - The softmax key-sum (fold tree + ones-matmul partition reduction) for
   one half is issued *after* the next half's first score chunks, so the
   PE never waits on the DVE/GPSIMD fold tree.  Reciprocals and the
   differential combine are issued only once the DRAM broadcast bounce
   has landed (combine split DVE/GPSIMD across query halves).
 - PSUM budget (8 banks): score ring 2x[128,1024] (4), AV accumulator
   [128,1024] (2), scratch (projections+sums) 2x[128,512] (2).
"""

import math
from contextlib import ExitStack

import ml_dtypes
import numpy as np

import concourse.bass as bass
import concourse.mybir as mybir
from concourse import bacc
import concourse.tile as tile
from concourse.bass_utils import run_bass_kernel_spmd

F32 = mybir.dt.float32
BF16 = mybir.dt.bfloat16
AF = mybir.ActivationFunctionType
ALU = mybir.AluOpType

N = 1024          # sequence length
D = 1024          # model dim
HG = 512          # head-group dims per core (4 heads x 128)
NHEAD = 4         # local heads per core
SCALE = 1.0 / math.sqrt(64.0)   # 1/sqrt(dk/2)
LAMBDA_INIT = 0.8

_BUILT = None     # cached Bass module -- building + compiling is expensive
LAST_RESULT = None  # BassKernelResults from the most recent run (for test.py)


def _build():
    nc = bacc.Bacc()

    # ---- DRAM I/O (per core) ----
    xqT = nc.dram_tensor("xqT", [D, N], BF16, kind="ExternalInput")
    xkT = nc.dram_tensor("xkT", [D, N], BF16, kind="ExternalInput")
    xvT = nc.dram_tensor("xvT", [D, N], BF16, kind="ExternalInput")  # col-permuted
    wqT = nc.dram_tensor("wqT", [D, HG], BF16, kind="ExternalInput")
    wkT = nc.dram_tensor("wkT", [D, HG], BF16, kind="ExternalInput")
    wvT = nc.dram_tensor("wvT", [D, HG], BF16, kind="ExternalInput")
    woT = nc.dram_tensor("woT", [HG, D], BF16, kind="ExternalInput")
    bq = nc.dram_tensor("bq", [HG], F32, kind="ExternalInput")
    bk = nc.dram_tensor("bk", [HG], F32, kind="ExternalInput")
    bv = nc.dram_tensor("bv", [HG], F32, kind="ExternalInput")
    lamneg = nc.dram_tensor("lamneg", [1, 1], F32, kind="ExternalInput")
    out = nc.dram_tensor("out", [N, D], F32, kind="ExternalOutput")

    with tile.TileContext(nc) as tc, ExitStack() as ctx:
        const = ctx.enter_context(tc.tile_pool(name="const", bufs=1))
        ones_bf = const.tile([128, 1], BF16, name="ones_bf")
        nc.vector.memset(ones_bf[:], 1.0)
        lamneg_bc = const.tile([128, 1], F32, name="lamneg_bc")
        bq_sb = const.tile([128, 4], F32, name="bq_sb")
        bk_sb = const.tile([128, 4], F32, name="bk_sb")
        bv_sb = const.tile([128, HG], F32, name="bv_sb")

        # Persistent activations / weights
        persist = ctx.enter_context(tc.tile_pool(name="persist", bufs=1))
        qt = [persist.tile([128, N], BF16, name=f"qt{t}") for t in range(4)]
        # qsw[j]: partitions 0:64 = qt[j][64:128], 64:128 = qt[j][0:64] -- lets
        # every (w, u) scores matmul read lhsT and rhs from the same base
        # partition.
        qsw = [persist.tile([128, N], BF16, name=f"qsw{t}") for t in range(4)]
        kt = [persist.tile([128, N], BF16, name=f"kt{t}") for t in range(4)]
        vv = [persist.tile([128, HG], BF16, name=f"vv{m}") for m in range(8)]
        oh = [persist.tile([128, N], BF16, name=f"oh{j}") for j in range(NHEAD)]
        wo_sb = [persist.tile([128, N], BF16, name=f"wo{t}") for t in range(4)]

        xw = ctx.enter_context(tc.tile_pool(name="xw", bufs=1))
        xs = {}
        ws = {}
        for nm in ("q", "k", "v"):
            for d in range(8):
                xst = xw.tile([128, N], BF16, name=f"x{nm}{d}")
                xs[(nm, d)] = xst
                wst = xw.tile([128, HG], BF16, name=f"w{nm}{d}")
                ws[(nm, d)] = wst

        # Input DMAs, in consumption order, spread across issue queues:
        # SP takes the big x tiles, ACT the weights, DVE the small consts
        # and wo -- serial per-issue cost (~0.6us) would otherwise delay
        # the later tiles by ~30us.
        nc.gpsimd.dma_start(out=bq_sb[:], in_=bq.rearrange("(t p) -> p t", p=128))
        nc.gpsimd.dma_start(out=bk_sb[:], in_=bk.rearrange("(t p) -> p t", p=128))
        nc.gpsimd.dma_start(out=bv_sb[:], in_=bv[None, :].to_broadcast([128, HG]))
        nc.gpsimd.dma_start(out=lamneg_bc[:],
                            in_=lamneg[0, :].partition_broadcast(128))
        for d in range(8):
            nc.scalar.dma_start(out=ws[("q", d)][:],
                                in_=wqT[d * 128:(d + 1) * 128, :])
            nc.gpsimd.dma_start(out=ws[("k", d)][:],
                                in_=wkT[d * 128:(d + 1) * 128, :])
        for nm, xd in (("q", xqT), ("k", xkT), ("v", xvT)):
            for d in range(8):
                nc.sync.dma_start(out=xs[(nm, d)][:],
                                  in_=xd[d * 128:(d + 1) * 128, :])
        for d in range(8):
            nc.sync.dma_start(out=ws[("v", d)][:],
                              in_=wvT[d * 128:(d + 1) * 128, :])
        for t in range(4):
            nc.scalar.dma_start(out=wo_sb[t][:],
                                in_=woT[t * 128:(t + 1) * 128, :])

        # Working pools
        etp = ctx.enter_context(tc.tile_pool(name="etp", bufs=9))
        coefp = ctx.enter_context(tc.tile_pool(name="coefp", bufs=4))
        fap = ctx.enter_context(tc.tile_pool(name="fap", bufs=3))
        ffp = ctx.enter_context(tc.tile_pool(name="ffp", bufs=2))
        bcastp = ctx.enter_context(tc.tile_pool(name="bcastp", bufs=3))
        drb = ctx.enter_context(tc.tile_pool(name="drb", bufs=4, space="DRAM"))
        osb = ctx.enter_context(tc.tile_pool(name="osb", bufs=3))
        tmpp = ctx.enter_context(tc.tile_pool(name="tmpp", bufs=1))
        ostg = ctx.enter_context(tc.tile_pool(name="ostg", bufs=2))

        # PSUM pools -- exactly 8 banks
        psp = ctx.enter_context(tc.tile_pool(name="psp", bufs=2, space="PSUM"))
        pops = ctx.enter_context(tc.tile_pool(name="pops", bufs=1, space="PSUM"))
        pscr = ctx.enter_context(tc.tile_pool(name="pscr", bufs=2, space="PSUM"))

        # ---- projection work generators (PE filler units) ----
        def qk_units(j, nm):
            dst = qt[j] if nm == "q" else kt[j]
            bias = bq_sb if nm == "q" else bk_sb
            for half in range(2):
                ps = pscr.tile([128, 512], F32, name=f"p{nm}{j}{half}", tag="scr")
                for d in range(8):
                    def mm(d=d, ps=ps, nm=nm, half=half, j=j):
                        nc.tensor.matmul(
                            ps[:],
                            ws[(nm, d)][:, j * 128:(j + 1) * 128],
                            xs[(nm, d)][:, half * 512:(half + 1) * 512],
                            start=(d == 0), stop=(d == 7),
                        )
                    yield ("mm", mm)

                def post(ps=ps, dst=dst, bias=bias, half=half, j=j, nm=nm):
                    nc.vector.tensor_scalar_add(
                        dst[:, half * 512:(half + 1) * 512], ps[:],
                        bias[:, j:j + 1],
                    )
                    if nm == "q" and half == 1:
                        nc.sync.dma_start(out=qsw[j][0:64, :],
                                          in_=qt[j][64:128, :])
                        nc.sync.dma_start(out=qsw[j][64:128, :],
                                          in_=qt[j][0:64, :])
                yield ("post", post)

        def v_units(j, mcs=(0, 1, 2, 3)):
            # V projection in consumption order (mi, mi+4) pairs
            for mc in mcs:
                ps = pscr.tile([128, 512], F32, name=f"pv{j}{mc}", tag="scr")
                for sub, mi in ((0, mc), (1, mc + 4)):
                    for d in range(8):
                        def mm(d=d, ps=ps, sub=sub, mi=mi, j=j):
                            nc.tensor.matmul(
                                ps[:, sub * 128:(sub + 1) * 128],
                                xs[("v", d)][:, mi * 128:(mi + 1) * 128],
                                ws[("v", d)][:, j * 128:(j + 1) * 128],
                                start=(d == 0), stop=(d == 7),
                            )
                        yield ("mm", mm)

                def post(ps=ps, mc=mc, j=j):
                    for sub, mi in ((0, mc), (1, mc + 4)):
                        nc.vector.tensor_tensor(
                            out=vv[mi][:, j * 128:(j + 1) * 128],
                            in0=ps[:, sub * 128:(sub + 1) * 128],
                            in1=bv_sb[:, j * 128:(j + 1) * 128], op=ALU.add,
                        )
                yield ("post", post)

        def chain(*gens):
            for g in gens:
                yield from g

        filler = {"gen": None, "buf": None}

        def pull(n):
            cnt = 0
            while True:
                u = filler["buf"]
                filler["buf"] = None
                if u is None:
                    g = filler["gen"]
                    u = next(g, None) if g is not None else None
                if u is None:
                    filler["gen"] = None
                    return
                kind, fn = u
                if kind == "post":
                    fn()
                    continue
                if cnt == n:
                    filler["buf"] = u
                    return
                fn()
                cnt += 1

        def drain():
            if filler["buf"] is not None:
                filler["buf"][1]()
                filler["buf"] = None
            g = filler["gen"]
            if g is None:
                return
            for kind, fn in g:
                fn()
            filler["gen"] = None

        # ---- prologue: Q/K projections for heads 0 and 1 (d-progressive,
        # consuming input DMAs as they land) ----
        filler["gen"] = chain(qk_units(0, "q"), qk_units(0, "k"))
        drain()

        # filler schedule: V of head j runs just-in-time inside head j.
        head_filler = {
            0: lambda: chain(v_units(0), qk_units(1, "q"), qk_units(1, "k"),
                             v_units(1)),
            1: lambda: chain(qk_units(2, "q"), qk_units(2, "k"), v_units(2)),
            2: lambda: chain(qk_units(3, "q"), qk_units(3, "k"),
                             v_units(3, mcs=(0, 1))),
            3: lambda: v_units(3, mcs=(2, 3)),
        }
        head_pull = {0: 8, 1: 6, 2: 6, 3: 8}

        # state carried between halves for the delayed sum reduction
        pend = {"ff": None, "bcx": None, "head": None, "half": None}
        parts = {j: {} for j in range(NHEAD)}   # per-head ot/bcx tiles
        combine_q = []
        recip0_q = []

        def emit_sps(u):
            """ones-matmul partition reduction + coef copy + broadcast DMAs
            for the pending half (u-th query half)."""
            ff = pend["ff"]
            hd, hf = pend["head"], pend["half"]
            sps = pscr.tile([1, 512], F32, name=f"sps{hd}{hf}{u}", tag="scr")
            nc.tensor.matmul(
                sps[:], ones_bf[:], ff[:, u * 512:(u + 1) * 512],
                start=True, stop=True,
            )
            cfs = coefp.tile([1, 512], F32, name=f"cfs{hd}{hf}{u}", tag="coef")
            nc.scalar.copy(cfs[:], sps[:])
            drs = drb.tile([1, 512], F32, name=f"drs{hd}{hf}{u}", tag="dr")
            nc.sync.dma_start(out=drs[:], in_=cfs[:])
            bcx = pend["bcx"]
            nc.sync.dma_start(
                out=bcx[:, u * 512:(u + 1) * 512],
                in_=drs[0, :].partition_broadcast(128),
            )
            if u == 1:
                parts[hd][f"bcx{hf}"] = bcx
                pend["ff"] = None
                if hf == 0:
                    recip0_q.append(hd)
                else:
                    combine_q.append(hd)

        def emit_recip0():
            while recip0_q:
                hd = recip0_q.pop(0)
                bcx0 = parts[hd]["bcx0"]
                nc.vector.reciprocal_approx_fast(out=bcx0[:], in_=bcx0[:])

        def emit_combine():
            """Recip of the second half + differential combine, split
            across DVE (u=0) and GPSIMD (u=1)."""
            if not combine_q:
                return
            hd = combine_q.pop(0)
            st = parts[hd]
            ot0, ot1 = st["ot0"], st["ot1"]
            bcx0, bcx1 = st["bcx0"], st["bcx1"]
            nc.vector.reciprocal_approx_fast(out=bcx1[:], in_=bcx1[:])
            # fold -lambda into the second-half normalizer so the combine is
            # plain tensor_tensor ops (Pool-ISA legal for the GPSIMD side)
            nc.vector.tensor_scalar_mul(bcx1[:], bcx1[:], lamneg_bc[:])
            for u in range(2):
                eng = nc.vector if u == 0 else nc.gpsimd
                sl = slice(u * 512, (u + 1) * 512)
                t1u = tmpp.tile([128, 512], F32, name=f"t1u{hd}{u}", tag=f"t1{u}")
                eng.tensor_tensor(
                    out=t1u[:], in0=ot0[:, sl], in1=bcx0[:, sl], op=ALU.mult)
                t2u = tmpp.tile([128, 512], F32, name=f"t2u{hd}{u}", tag=f"t2{u}")
                eng.tensor_tensor(
                    out=t2u[:], in0=ot1[:, sl], in1=bcx1[:, sl], op=ALU.mult)
                eng.tensor_tensor(
                    out=oh[hd][:, sl], in0=t1u[:], in1=t2u[:], op=ALU.add)

        # ---- attention per head, pipelined with projection filler ----
        for j in range(NHEAD):
            drain()
            filler["gen"] = head_filler[j]()
            for half, lo in ((0, 0), (1, 512)):
                ets = []
                ops = pops.tile([128, N], F32, name=f"ops{j}{half}", tag="ops")
                fa = [None] * 4
                fb = None

                def av(c, start, stop, j=j, ops=ops, ets=ets):
                    mi = (c % 2) * 4 + (c // 2)
                    for u in range(2):
                        nc.tensor.matmul(
                            ops[:, u * 512:(u + 1) * 512],
                            vv[mi][:, j * 128:(j + 1) * 128],
                            ets[c][:, u * 512:(u + 1) * 512],
                            start=start, stop=stop,
                        )

                for c in range(8):
                    mc, w = c // 2, c % 2
                    sp = psp.tile([128, N], F32, name=f"sp{j}{half}{c}", tag="sp")
                    for u in range(2):
                        qsrc = qt[j] if u == w else qsw[j]
                        nc.tensor.matmul(
                            sp[:, u * 512:(u + 1) * 512],
                            kt[j][w * 64:(w + 1) * 64,
                                  lo + mc * 128:lo + (mc + 1) * 128],
                            qsrc[w * 64:(w + 1) * 64, lo:lo + 512],
                            start=True, stop=True,
                        )
                    et = etp.tile([128, N], BF16, name=f"et{j}{half}{c}", tag="et")
                    nc.scalar.activation(et[:], sp[:], AF.Exp, scale=SCALE)
                    ets.append(et)

                    # delayed partition-sums of the previous half
                    if pend["ff"] is not None:
                        if c == 2:
                            emit_sps(0)
                        elif c == 5:
                            emit_sps(1)

                    pull(16 if c == 0 else head_pull[j])
                    if c >= 1:
                        av(c - 1, start=(c == 1), stop=False)

                    # fold tree: pairs (0,1) DVE, (2,3) GPSIMD, (4,5) GPSIMD,
                    # (6,7) DVE; inner combines scheduled so no engine ever
                    # head-of-line blocks on a slow producer.
                    if c == 1:
                        fa[0] = fap.tile([128, N], BF16, name=f"fa{j}{half}0",
                                         tag="fa")
                        nc.vector.tensor_tensor(out=fa[0][:], in0=ets[0][:],
                                                in1=ets[1][:], op=ALU.add)
                    elif c == 3:
                        fa[1] = fap.tile([128, N], BF16, name=f"fa{j}{half}1",
                                         tag="fa")
                        nc.gpsimd.tensor_tensor(out=fa[1][:], in0=ets[2][:],
                                                in1=ets[3][:], op=ALU.add)
                    elif c == 5:
                        fa[2] = fap.tile([128, N], BF16, name=f"fa{j}{half}2",
                                         tag="fa")
                        nc.gpsimd.tensor_tensor(out=fa[2][:], in0=ets[4][:],
                                                in1=ets[5][:], op=ALU.add)
                        fb = fap.tile([128, N], BF16, name=f"fb{j}{half}",
                                      tag="fb")
                        nc.gpsimd.tensor_tensor(out=fb[:], in0=fa[0][:],
                                                in1=fa[1][:], op=ALU.add)
                    elif c == 7:
                        fa[3] = fap.tile([128, N], BF16, name=f"fa{j}{half}3",
                                         tag="fa")
                        nc.vector.tensor_tensor(out=fa[3][:], in0=ets[6][:],
                                                in1=ets[7][:], op=ALU.add)
                        fc = fap.tile([128, N], BF16, name=f"fc{j}{half}",
                                      tag="fc")
                        nc.vector.tensor_tensor(out=fc[:], in0=fa[2][:],
                                                in1=fa[3][:], op=ALU.add)
                        ff = ffp.tile([128, N], BF16, name=f"ff{j}{half}",
                                      tag="ff")
                        nc.vector.tensor_tensor(out=ff[:], in0=fb[:],
                                                in1=fc[:], op=ALU.add)
                        emit_recip0()
                        emit_combine()

                av(7, start=False, stop=True)
                ot = osb.tile([128, N], F32, name=f"ot{j}{half}", tag="ot")
                nc.vector.tensor_copy(out=ot[:], in_=ops[:])
                parts[j][f"ot{half}"] = ot

                bcx = bcastp.tile([128, N], F32, name=f"bcx{j}{half}", tag="bc")
                pend.update(ff=ff, bcx=bcx, head=j, half=half)

        # ---- tail: last half's sums, final combine, output projection ----
        drain()
        emit_sps(0)
        emit_sps(1)

        ps_of = {}
        tags = ["sp", "sp", "ops", "scr", "scr"]
        pools = {"sp": psp, "ops": pops, "scr": pscr}

        def alloc_out_ps(idx, k):
            tg = tags[k % 5]
            ps = pools[tg].tile([128, 512], F32, name=f"po{idx}", tag=tg)
            ps_of[idx] = ps

        def outproj_mm(idx, jlist, start):
            nci, half = idx // 2, idx % 2
            ps = ps_of[idx]
            for jj in jlist:
                nc.tensor.matmul(
                    ps[:],
                    oh[jj][:, nci * 128:(nci + 1) * 128],
                    wo_sb[jj][:, half * 512:(half + 1) * 512],
                    start=(start and jj == jlist[0]), stop=(jj == 3),
                )

        def outproj_fin(idx):
            nci, half = idx // 2, idx % 2
            ps = ps_of[idx]
            stg = ostg.tile([128, 512], F32, name=f"stg{idx}", tag="og")
            eng = nc.vector if idx % 2 == 0 else nc.scalar
            if idx % 2 == 0:
                nc.vector.tensor_copy(out=stg[:], in_=ps[:])
            else:
                nc.scalar.copy(stg[:], ps[:])
            if idx >= 12:
                for q in range(2):
                    nc.sync.dma_start(
                        out=out[nci * 128:(nci + 1) * 128,
                                half * 512 + q * 256:half * 512 + (q + 1) * 256],
                        in_=stg[:, q * 256:(q + 1) * 256],
                    )
            else:
                nc.sync.dma_start(
                    out=out[nci * 128:(nci + 1) * 128,
                            half * 512:(half + 1) * 512],
                    in_=stg[:],
                )

        # wave A: pre-accumulate heads 0..2 while head 3's sum-bounce is in
        # flight.
        for k, idx in enumerate(range(5)):
            alloc_out_ps(idx, k)
            outproj_mm(idx, [0, 1, 2], start=True)
        emit_combine()
        for idx in range(5):
            outproj_mm(idx, [3], start=False)
            outproj_fin(idx)
        for k, idx in enumerate(range(5, 16)):
            alloc_out_ps(idx, 5 + k)
            outproj_mm(idx, [0, 1, 2, 3], start=True)
            outproj_fin(idx)

    if not nc.is_finalized():
        nc.finalize()
    return nc


def _get_built():
    global _BUILT
    if _BUILT is None:
        _BUILT = _build()
    return _BUILT


def kernel(**inputs):
    inp = {k: np.asarray(v) for k, v in inputs.items()}
    q_, k_, v_ = inp["query"], inp["key"], inp["value"]
    Wq, Wk, Wv, Wo = inp["Wq"], inp["Wk"], inp["Wv"], inp["Wo"]
    bq_, bk_, bv_, bo_ = inp["bq"], inp["bk"], inp["bv"], inp["bo"]
    B = q_.shape[0]

    lam = (np.exp(np.sum(inp["lambda_q1"].astype(np.float64) * inp["lambda_k1"].astype(np.float64)))
           - np.exp(np.sum(inp["lambda_q2"].astype(np.float64) * inp["lambda_k2"].astype(np.float64)))
           + LAMBDA_INIT)

    # value-row permutation: xv'[w*512 + m] = xv[2m + w]
    permv = np.arange(N).reshape(512, 2).T.reshape(-1)  # index i'=w*512+m -> 2m+w

    in_maps = []
    for c in range(8):
        b, g = c // 2, c % 2
        sl = slice(g * HG, (g + 1) * HG)
        bf = ml_dtypes.bfloat16
        in_maps.append({
            "xqT": np.ascontiguousarray(q_[b].T).astype(bf),
            "xkT": np.ascontiguousarray(k_[b].T).astype(bf),
            "xvT": np.ascontiguousarray(v_[b][permv].T).astype(bf),
            "wqT": np.ascontiguousarray(Wq[sl, :].T).astype(bf),
            "wkT": np.ascontiguousarray(Wk[sl, :].T).astype(bf),
            "wvT": np.ascontiguousarray(Wv[sl, :].T).astype(bf),
            "woT": np.ascontiguousarray(Wo[:, sl].T).astype(bf),
            "bq": np.ascontiguousarray(bq_[sl]),
            "bk": np.ascontiguousarray(bk_[sl]),
            "bv": np.ascontiguousarray(bv_[sl]),
            "lamneg": np.array([[-lam]], dtype=np.float32),
        })

    nc = _get_built()
    res = run_bass_kernel_spmd(nc, in_maps, core_ids=list(range(8)))
    global LAST_RESULT
    LAST_RESULT = res

    out = np.zeros((B, N, D), np.float32)
    for b in range(B):
        tot = res.results[2 * b]["out"] + res.results[2 * b + 1]["out"]
        # undo n' = (u, n) row order -> n2 = 2n + u
        out[b] = tot.reshape(2, 512, D).transpose(1, 0, 2).reshape(N, D) + bo_
    return out


# revision 16
# speedup vs baseline: 1.1577x; 1.0310x over previous
"""Differential multi-headed attention on 8 Trainium2 NeuronCores.

Sharding: core c = (batch b = c // 2, head-group g = c % 2).  Each core
computes 4 of the 8 differential heads for one batch element, including
the Q/K/V projections restricted to its 512 output dims, the differential
attention, and a partial output projection.  The host sums the two
partial outputs per batch (the "all-reduce"), un-permutes rows, and adds
the output bias.

Device schedule notes (v3):
 - The TRN2 PE has a p-state ramp (0.65 -> 1.2 -> 2.4 GHz over 3us of
   *continuous* execution; any idle gap resets it).  The kernel is a
   software pipeline that keeps the PE stream gap-free: projection
   matmuls for later heads are interleaved as filler between the
   attention matmuls of the current head (V of head j runs just-in-time
   inside head j itself, so even the last head has filler).
 - Input DMAs are issued from the GPSIMD queue (25ns/issue vs 565ns on
   SP), in consumption order, with x tiles split into column halves so
   the d-progressive projection groups start as soon as pieces land.
 - The softmax key-sum (fold tree + ones-matmul partition reduction) for
   one half is issued *after* the next half's first score chunks, so the
   PE never waits on the DVE/GPSIMD fold tree.  Reciprocals and the
   differential combine are issued only once the DRAM broadcast bounce
   has landed (combine split DVE/GPSIMD across query halves).
 - PSUM budget (8 banks): score ring 2x[128,1024] (4), AV accumulator
   [128,1024] (2), scratch (projections+sums) 2x[128,512] (2).
"""

import math
from contextlib import ExitStack

import ml_dtypes
import numpy as np

import concourse.bass as bass
import concourse.mybir as mybir
from concourse import bacc
import concourse.tile as tile
from concourse.bass_utils import run_bass_kernel_spmd

F32 = mybir.dt.float32
BF16 = mybir.dt.bfloat16
AF = mybir.ActivationFunctionType
ALU = mybir.AluOpType

N = 1024          # sequence length
D = 1024          # model dim
HG = 512          # head-group dims per core (4 heads x 128)
NHEAD = 4         # local heads per core
SCALE = 1.0 / math.sqrt(64.0)   # 1/sqrt(dk/2)
LAMBDA_INIT = 0.8

_BUILT = None     # cached Bass module -- building + compiling is expensive
LAST_RESULT = None  # BassKernelResults from the most recent run (for test.py)


def _build():
    nc = bacc.Bacc()

    # ---- DRAM I/O (per core) ----
    xqT = nc.dram_tensor("xqT", [D, N], BF16, kind="ExternalInput")
    xkT = nc.dram_tensor("xkT", [D, N], BF16, kind="ExternalInput")
    xvT = nc.dram_tensor("xvT", [D, N], BF16, kind="ExternalInput")  # col-permuted
    wqT = nc.dram_tensor("wqT", [D, HG], BF16, kind="ExternalInput")
    wkT = nc.dram_tensor("wkT", [D, HG], BF16, kind="ExternalInput")
    wvT = nc.dram_tensor("wvT", [D, HG], BF16, kind="ExternalInput")
    woT = nc.dram_tensor("woT", [HG, D], BF16, kind="ExternalInput")
    bq = nc.dram_tensor("bq", [HG], F32, kind="ExternalInput")
    bk = nc.dram_tensor("bk", [HG], F32, kind="ExternalInput")
    bv = nc.dram_tensor("bv", [HG], F32, kind="ExternalInput")
    lamneg = nc.dram_tensor("lamneg", [1, 1], F32, kind="ExternalInput")
    out = nc.dram_tensor("out", [N, D], BF16, kind="ExternalOutput")

    with tile.TileContext(nc) as tc, ExitStack() as ctx:
        const = ctx.enter_context(tc.tile_pool(name="const", bufs=1))
        ones_bf = const.tile([128, 1], BF16, name="ones_bf")
        nc.vector.memset(ones_bf[:], 1.0)
        lamneg_bc = const.tile([128, 1], F32, name="lamneg_bc")
        bq_sb = const.tile([128, 4], F32, name="bq_sb")
        bk_sb = const.tile([128, 4], F32, name="bk_sb")
        bv_sb = const.tile([128, HG], F32, name="bv_sb")

        # Persistent activations / weights
        persist = ctx.enter_context(tc.tile_pool(name="persist", bufs=1))
        qt = [persist.tile([128, N], BF16, name=f"qt{t}") for t in range(4)]
        # qsw[j]: partitions 0:64 = qt[j][64:128], 64:128 = qt[j][0:64] -- lets
        # every (w, u) scores matmul read lhsT and rhs from the same base
        # partition.
        qsw = [persist.tile([128, N], BF16, name=f"qsw{t}") for t in range(4)]
        kt = [persist.tile([128, N], BF16, name=f"kt{t}") for t in range(4)]
        vv = [persist.tile([128, HG], BF16, name=f"vv{m}") for m in range(8)]
        oh = [persist.tile([128, N], BF16, name=f"oh{j}") for j in range(NHEAD)]
        wo_sb = [persist.tile([128, N], BF16, name=f"wo{t}") for t in range(4)]

        xw = ctx.enter_context(tc.tile_pool(name="xw", bufs=1))
        xs = {}
        ws = {}
        for nm in ("q", "k", "v"):
            for d in range(8):
                if nm in ("q", "k"):
                    xst = xw.tile([128, N], BF16, name=f"x{nm}{d}",
                                  tag=f"rx{nm}", bufs=8)
                else:
                    xst = xw.tile([128, N], BF16, name=f"x{nm}{d}")
                xs[(nm, d)] = xst
                wst = xw.tile([128, HG], BF16, name=f"w{nm}{d}")
                ws[(nm, d)] = wst

        # Input DMAs, in consumption order, spread across issue queues:
        # SP takes the big x tiles, ACT the weights, DVE the small consts
        # and wo -- serial per-issue cost (~0.6us) would otherwise delay
        # the later tiles by ~30us.
        nc.gpsimd.dma_start(out=bq_sb[:], in_=bq.rearrange("(t p) -> p t", p=128))
        nc.gpsimd.dma_start(out=bk_sb[:], in_=bk.rearrange("(t p) -> p t", p=128))
        nc.gpsimd.dma_start(out=bv_sb[:], in_=bv[None, :].to_broadcast([128, HG]))
        nc.gpsimd.dma_start(out=lamneg_bc[:],
                            in_=lamneg[0, :].partition_broadcast(128))
        for d in range(8):
            nc.scalar.dma_start(out=ws[("q", d)][:],
                                in_=wqT[d * 128:(d + 1) * 128, :])
            nc.gpsimd.dma_start(out=ws[("k", d)][:],
                                in_=wkT[d * 128:(d + 1) * 128, :])
        for nm, xd in (("q", xqT), ("k", xkT), ("v", xvT)):
            for d in range(8):
                nc.sync.dma_start(out=xs[(nm, d)][:],
                                  in_=xd[d * 128:(d + 1) * 128, :])
        for d in range(8):
            nc.sync.dma_start(out=ws[("v", d)][:],
                              in_=wvT[d * 128:(d + 1) * 128, :])
        for t in range(4):
            nc.scalar.dma_start(out=wo_sb[t][:],
                                in_=woT[t * 128:(t + 1) * 128, :])

        # Working pools
        etp = ctx.enter_context(tc.tile_pool(name="etp", bufs=8))
        coefp = ctx.enter_context(tc.tile_pool(name="coefp", bufs=4))
        fap = ctx.enter_context(tc.tile_pool(name="fap", bufs=3))
        ffp = ctx.enter_context(tc.tile_pool(name="ffp", bufs=2))
        bcastp = ctx.enter_context(tc.tile_pool(name="bcastp", bufs=3))
        drb = ctx.enter_context(tc.tile_pool(name="drb", bufs=4, space="DRAM"))
        osb = ctx.enter_context(tc.tile_pool(name="osb", bufs=3))
        tmpp = ctx.enter_context(tc.tile_pool(name="tmpp", bufs=1))
        ostg = ctx.enter_context(tc.tile_pool(name="ostg", bufs=8))

        # PSUM pools -- exactly 8 banks
        psp = ctx.enter_context(tc.tile_pool(name="psp", bufs=2, space="PSUM"))
        pops = ctx.enter_context(tc.tile_pool(name="pops", bufs=1, space="PSUM"))
        pscr = ctx.enter_context(tc.tile_pool(name="pscr", bufs=2, space="PSUM"))

        # ---- projection work generators (PE filler units) ----
        def qk_units(j, nm):
            dst = qt[j] if nm == "q" else kt[j]
            bias = bq_sb if nm == "q" else bk_sb
            for half in range(2):
                ps = pscr.tile([128, 512], F32, name=f"p{nm}{j}{half}", tag="scr")
                for d in range(8):
                    def mm(d=d, ps=ps, nm=nm, half=half, j=j):
                        nc.tensor.matmul(
                            ps[:],
                            ws[(nm, d)][:, j * 128:(j + 1) * 128],
                            xs[(nm, d)][:, half * 512:(half + 1) * 512],
                            start=(d == 0), stop=(d == 7),
                        )
                    yield ("mm", mm)

                def post(ps=ps, dst=dst, bias=bias, half=half, j=j, nm=nm):
                    nc.vector.tensor_scalar_add(
                        dst[:, half * 512:(half + 1) * 512], ps[:],
                        bias[:, j:j + 1],
                    )
                    if nm == "q" and half == 1:
                        nc.sync.dma_start(out=qsw[j][0:64, :],
                                          in_=qt[j][64:128, :])
                        nc.sync.dma_start(out=qsw[j][64:128, :],
                                          in_=qt[j][0:64, :])
                yield ("post", post)

        def v_units(j, mcs=(0, 1, 2, 3)):
            # V projection in consumption order (mi, mi+4) pairs
            for mc in mcs:
                ps = pscr.tile([128, 512], F32, name=f"pv{j}{mc}", tag="scr")
                for sub, mi in ((0, mc), (1, mc + 4)):
                    for d in range(8):
                        def mm(d=d, ps=ps, sub=sub, mi=mi, j=j):
                            nc.tensor.matmul(
                                ps[:, sub * 128:(sub + 1) * 128],
                                xs[("v", d)][:, mi * 128:(mi + 1) * 128],
                                ws[("v", d)][:, j * 128:(j + 1) * 128],
                                start=(d == 0), stop=(d == 7),
                            )
                        yield ("mm", mm)

                def post(ps=ps, mc=mc, j=j):
                    for sub, mi in ((0, mc), (1, mc + 4)):
                        nc.vector.tensor_tensor(
                            out=vv[mi][:, j * 128:(j + 1) * 128],
                            in0=ps[:, sub * 128:(sub + 1) * 128],
                            in1=bv_sb[:, j * 128:(j + 1) * 128], op=ALU.add,
                        )
                yield ("post", post)

        def chain(*gens):
            for g in gens:
                yield from g

        filler = {"gen": None, "buf": None}

        def pull(n):
            cnt = 0
            while True:
                u = filler["buf"]
                filler["buf"] = None
                if u is None:
                    g = filler["gen"]
                    u = next(g, None) if g is not None else None
                if u is None:
                    filler["gen"] = None
                    return
                kind, fn = u
                if kind == "post":
                    fn()
                    continue
                if cnt == n:
                    filler["buf"] = u
                    return
                fn()
                cnt += 1

        def drain():
            if filler["buf"] is not None:
                filler["buf"][1]()
                filler["buf"] = None
            g = filler["gen"]
            if g is None:
                return
            for kind, fn in g:
                fn()
            filler["gen"] = None

        # ---- prologue: Q/K projections for heads 0 and 1 (d-progressive,
        # consuming input DMAs as they land) ----
        filler["gen"] = chain(qk_units(0, "q"), qk_units(0, "k"))
        drain()

        # filler schedule: V of head j runs just-in-time inside head j.
        head_filler = {
            0: lambda: chain(v_units(0), qk_units(1, "q"), qk_units(1, "k"),
                             v_units(1)),
            1: lambda: chain(qk_units(2, "q"), qk_units(2, "k"), v_units(2)),
            2: lambda: chain(qk_units(3, "q"), qk_units(3, "k"),
                             v_units(3, mcs=(0, 1))),
            3: lambda: v_units(3, mcs=(2, 3)),
        }
        head_pull = {0: 8, 1: 6, 2: 6, 3: 8}

        # state carried between halves for the delayed sum reduction
        pend = {"ff": None, "bcx": None, "head": None, "half": None}
        parts = {j: {} for j in range(NHEAD)}   # per-head ot/bcx tiles
        combine_q = []
        recip0_q = []

        def emit_sps(u):
            """ones-matmul partition reduction + coef copy + broadcast DMAs
            for the pending half (u-th query half)."""
            ff = pend["ff"]
            hd, hf = pend["head"], pend["half"]
            sps = pscr.tile([1, 512], F32, name=f"sps{hd}{hf}{u}", tag="scr")
            nc.tensor.matmul(
                sps[:], ones_bf[:], ff[:, u * 512:(u + 1) * 512],
                start=True, stop=True,
            )
            cfs = coefp.tile([1, 512], F32, name=f"cfs{hd}{hf}{u}", tag="coef")
            nc.scalar.copy(cfs[:], sps[:])
            drs = drb.tile([1, 512], F32, name=f"drs{hd}{hf}{u}", tag="dr")
            nc.sync.dma_start(out=drs[:], in_=cfs[:])
            bcx = pend["bcx"]
            nc.sync.dma_start(
                out=bcx[:, u * 512:(u + 1) * 512],
                in_=drs[0, :].partition_broadcast(128),
            )
            if u == 1:
                parts[hd][f"bcx{hf}"] = bcx
                pend["ff"] = None
                if hf == 0:
                    recip0_q.append(hd)
                else:
                    combine_q.append(hd)

        def emit_recip0():
            while recip0_q:
                hd = recip0_q.pop(0)
                bcx0 = parts[hd]["bcx0"]
                nc.vector.reciprocal_approx_fast(out=bcx0[:], in_=bcx0[:])

        def emit_combine():
            """Recip of the second half + differential combine, split
            across DVE (u=0) and GPSIMD (u=1)."""
            if not combine_q:
                return
            hd = combine_q.pop(0)
            st = parts[hd]
            ot0, ot1 = st["ot0"], st["ot1"]
            bcx0, bcx1 = st["bcx0"], st["bcx1"]
            nc.vector.reciprocal_approx_fast(out=bcx1[:], in_=bcx1[:])
            # fold -lambda into the second-half normalizer so the combine is
            # plain tensor_tensor ops (Pool-ISA legal for the GPSIMD side)
            nc.vector.tensor_scalar_mul(bcx1[:], bcx1[:], lamneg_bc[:])
            for u in range(2):
                eng = nc.vector if u == 0 else nc.gpsimd
                sl = slice(u * 512, (u + 1) * 512)
                t1u = tmpp.tile([128, 512], F32, name=f"t1u{hd}{u}", tag="t1")
                eng.tensor_tensor(
                    out=t1u[:], in0=ot0[:, sl], in1=bcx0[:, sl], op=ALU.mult)
                t2u = tmpp.tile([128, 512], F32, name=f"t2u{hd}{u}", tag="t2")
                eng.tensor_tensor(
                    out=t2u[:], in0=ot1[:, sl], in1=bcx1[:, sl], op=ALU.mult)
                eng.tensor_tensor(
                    out=oh[hd][:, sl], in0=t1u[:], in1=t2u[:], op=ALU.add)

        # ---- output-projection partials (heads 0..2) ----
        pstg_of = {}
        pp_n = {"n": 0}

        def partial_units(tiles, rota=None):
            for t in tiles:
                nci, phalf = t // 2, t % 2
                if rota is None:
                    ps = pscr.tile([128, 512], F32, name=f"pp{t}", tag="scr")
                else:
                    tg, pool = rota[pp_n["n"] % 5]
                    pp_n["n"] += 1
                    ps = pool.tile([128, 512], F32, name=f"pp{t}", tag=tg)
                for jj in (0, 1, 2):
                    def mm(jj=jj, ps=ps, nci=nci, phalf=phalf):
                        nc.tensor.matmul(
                            ps[:],
                            oh[jj][:, nci * 128:(nci + 1) * 128],
                            wo_sb[jj][:, phalf * 512:(phalf + 1) * 512],
                            start=(jj == 0), stop=(jj == 2),
                        )
                    yield ("mm", mm)

                def post(ps=ps, t=t):
                    rtag = "rxq" if t < 8 else "rxk"
                    pstg = xw.tile([128, 512], F32, name=f"pstg{t}",
                                   tag=rtag, bufs=8)
                    if t % 2 == 0:
                        nc.vector.tensor_copy(out=pstg[:], in_=ps[:])
                    else:
                        nc.scalar.copy(pstg[:], ps[:])
                    pstg_of[t] = pstg
                yield ("post", post)

        # ---- attention per head, pipelined with projection filler ----
        for j in range(NHEAD):
            drain()
            filler["gen"] = head_filler[j]()
            for half, lo in ((0, 0), (1, 512)):
                ets = []
                ops = pops.tile([128, N], F32, name=f"ops{j}{half}", tag="ops")
                fa = [None] * 4
                fb = None

                def av(c, start, stop, j=j, ops=ops, ets=ets):
                    mi = (c % 2) * 4 + (c // 2)
                    for u in range(2):
                        nc.tensor.matmul(
                            ops[:, u * 512:(u + 1) * 512],
                            vv[mi][:, j * 128:(j + 1) * 128],
                            ets[c][:, u * 512:(u + 1) * 512],
                            start=start, stop=stop,
                        )

                for c in range(8):
                    mc, w = c // 2, c % 2
                    sp = psp.tile([128, N], F32, name=f"sp{j}{half}{c}", tag="sp")
                    for u in range(2):
                        qsrc = qt[j] if u == w else qsw[j]
                        nc.tensor.matmul(
                            sp[:, u * 512:(u + 1) * 512],
                            kt[j][w * 64:(w + 1) * 64,
                                  lo + mc * 128:lo + (mc + 1) * 128],
                            qsrc[w * 64:(w + 1) * 64, lo:lo + 512],
                            start=True, stop=True,
                        )
                    et = etp.tile([128, N], BF16, name=f"et{j}{half}{c}", tag="et")
                    nc.scalar.activation(et[:], sp[:], AF.Exp, scale=SCALE)
                    ets.append(et)

                    # delayed partition-sums of the previous half
                    if pend["ff"] is not None:
                        if c == 2:
                            emit_sps(0)
                        elif c == 5:
                            emit_sps(1)
                    if j == 3 and half == 1 and c == 2:
                        drain()
                        filler["gen"] = partial_units(range(8))

                    pull(16 if c == 0 else head_pull[j])
                    if c >= 1:
                        av(c - 1, start=(c == 1), stop=False)

                    # fold tree: pairs (0,1) DVE, (2,3) GPSIMD, (4,5) GPSIMD,
                    # (6,7) DVE; inner combines scheduled so no engine ever
                    # head-of-line blocks on a slow producer.
                    if c == 1:
                        fa[0] = fap.tile([128, N], BF16, name=f"fa{j}{half}0",
                                         tag="fa")
                        nc.vector.tensor_tensor(out=fa[0][:], in0=ets[0][:],
                                                in1=ets[1][:], op=ALU.add)
                    elif c == 3:
                        fa[1] = fap.tile([128, N], BF16, name=f"fa{j}{half}1",
                                         tag="fa")
                        nc.gpsimd.tensor_tensor(out=fa[1][:], in0=ets[2][:],
                                                in1=ets[3][:], op=ALU.add)
                    elif c == 5:
                        fa[2] = fap.tile([128, N], BF16, name=f"fa{j}{half}2",
                                         tag="fa")
                        nc.gpsimd.tensor_tensor(out=fa[2][:], in0=ets[4][:],
                                                in1=ets[5][:], op=ALU.add)
                        fb = fap.tile([128, N], BF16, name=f"fb{j}{half}",
                                      tag="fb", bufs=2)
                        nc.gpsimd.tensor_tensor(out=fb[:], in0=fa[0][:],
                                                in1=fa[1][:], op=ALU.add)
                    elif c == 7:
                        fa[3] = fap.tile([128, N], BF16, name=f"fa{j}{half}3",
                                         tag="fa")
                        nc.vector.tensor_tensor(out=fa[3][:], in0=ets[6][:],
                                                in1=ets[7][:], op=ALU.add)
                        fc = fap.tile([128, N], BF16, name=f"fc{j}{half}",
                                      tag="fc", bufs=2)
                        nc.vector.tensor_tensor(out=fc[:], in0=fa[2][:],
                                                in1=fa[3][:], op=ALU.add)
                        ff = ffp.tile([128, N], BF16, name=f"ff{j}{half}",
                                      tag="ff")
                        nc.vector.tensor_tensor(out=ff[:], in0=fb[:],
                                                in1=fc[:], op=ALU.add)
                        emit_recip0()
                        emit_combine()

                av(7, start=False, stop=True)
                ot = osb.tile([128, N], F32, name=f"ot{j}{half}", tag="ot")
                nc.vector.tensor_copy(out=ot[:], in_=ops[:])
                parts[j][f"ot{half}"] = ot

                bcx = bcastp.tile([128, N], F32, name=f"bcx{j}{half}", tag="bc")
                pend.update(ff=ff, bcx=bcx, head=j, half=half)

        # ---- tail: last half's sums, final combine, output projection ----
        drain()
        emit_sps(0)
        emit_sps(1)

        rota = [("sp", psp), ("sp", psp), ("ops", pops),
                ("scr", pscr), ("scr", pscr)]

        def next_slot():
            tg, pool = rota[pp_n["n"] % 5]
            pp_n["n"] += 1
            return pool, tg

        def fin_tile(t, ps, add_pstg):
            nci, phalf = t // 2, t % 2
            stg = ostg.tile([128, 512], BF16, name=f"stg{t}", tag="og")
            if add_pstg:
                nc.vector.tensor_tensor(out=stg[:], in0=ps[:],
                                        in1=pstg_of[t][:], op=ALU.add)
            elif t % 2 == 0:
                nc.vector.tensor_copy(out=stg[:], in_=ps[:])
            else:
                nc.scalar.copy(stg[:], ps[:])
            dql = (nc.sync, nc.scalar, nc.gpsimd)[t % 3]
            dql.dma_start(
                out=out[nci * 128:(nci + 1) * 128,
                        phalf * 512:(phalf + 1) * 512],
                in_=stg[:],
            )

        def op_mm(t, ps, jlist, start, stop):
            nci, phalf = t // 2, t % 2
            for jj in jlist:
                nc.tensor.matmul(
                    ps[:],
                    oh[jj][:, nci * 128:(nci + 1) * 128],
                    wo_sb[jj][:, phalf * 512:(phalf + 1) * 512],
                    start=(start and jj == jlist[0]),
                    stop=(stop and jj == jlist[-1]),
                )

        # open-group partials for tiles 8..12 fill the PE while head 3's
        # sum-bounce and combine are in flight
        open_ps = {}
        for t in range(8, 13):
            pool, tg = next_slot()
            ps = pool.tile([128, 512], F32, name=f"pp{t}", tag=tg)
            open_ps[t] = ps
            op_mm(t, ps, [0, 1, 2], start=True, stop=False)

        emit_combine()

        for t in range(8, 13):
            op_mm(t, open_ps[t], [3], start=False, stop=True)
            fin_tile(t, open_ps[t], add_pstg=False)
        for t in range(0, 8):
            pool, tg = next_slot()
            ps = pool.tile([128, 512], F32, name=f"pf{t}", tag=tg)
            op_mm(t, ps, [3], start=True, stop=True)
            fin_tile(t, ps, add_pstg=True)
        for t in range(13, 16):
            pool, tg = next_slot()
            ps = pool.tile([128, 512], F32, name=f"pf{t}", tag=tg)
            op_mm(t, ps, [0, 1, 2, 3], start=True, stop=True)
            fin_tile(t, ps, add_pstg=False)

    if not nc.is_finalized():
        nc.finalize()
    return nc


def _get_built():
    global _BUILT
    if _BUILT is None:
        _BUILT = _build()
    return _BUILT


def kernel(**inputs):
    inp = {k: np.asarray(v) for k, v in inputs.items()}
    q_, k_, v_ = inp["query"], inp["key"], inp["value"]
    Wq, Wk, Wv, Wo = inp["Wq"], inp["Wk"], inp["Wv"], inp["Wo"]
    bq_, bk_, bv_, bo_ = inp["bq"], inp["bk"], inp["bv"], inp["bo"]
    B = q_.shape[0]

    lam = (np.exp(np.sum(inp["lambda_q1"].astype(np.float64) * inp["lambda_k1"].astype(np.float64)))
           - np.exp(np.sum(inp["lambda_q2"].astype(np.float64) * inp["lambda_k2"].astype(np.float64)))
           + LAMBDA_INIT)

    # value-row permutation: xv'[w*512 + m] = xv[2m + w]
    permv = np.arange(N).reshape(512, 2).T.reshape(-1)  # index i'=w*512+m -> 2m+w

    in_maps = []
    for c in range(8):
        b, g = c // 2, c % 2
        sl = slice(g * HG, (g + 1) * HG)
        bf = ml_dtypes.bfloat16
        in_maps.append({
            "xqT": np.ascontiguousarray(q_[b].T).astype(bf),
            "xkT": np.ascontiguousarray(k_[b].T).astype(bf),
            "xvT": np.ascontiguousarray(v_[b][permv].T).astype(bf),
            "wqT": np.ascontiguousarray(Wq[sl, :].T).astype(bf),
            "wkT": np.ascontiguousarray(Wk[sl, :].T).astype(bf),
            "wvT": np.ascontiguousarray(Wv[sl, :].T).astype(bf),
            "woT": np.ascontiguousarray(Wo[:, sl].T).astype(bf),
            "bq": np.ascontiguousarray(bq_[sl]),
            "bk": np.ascontiguousarray(bk_[sl]),
            "bv": np.ascontiguousarray(bv_[sl]),
            "lamneg": np.array([[-lam]], dtype=np.float32),
        })

    nc = _get_built()
    res = run_bass_kernel_spmd(nc, in_maps, core_ids=list(range(8)))
    global LAST_RESULT
    LAST_RESULT = res

    out = np.zeros((B, N, D), np.float32)
    for b in range(B):
        tot = (res.results[2 * b]["out"].astype(np.float32)
               + res.results[2 * b + 1]["out"].astype(np.float32))
        # undo n' = (u, n) row order -> n2 = 2n + u
        out[b] = tot.reshape(2, 512, D).transpose(1, 0, 2).reshape(N, D) + bo_
    return out
